# revision 5
# baseline (speedup 1.0000x reference)
"""Bass/Trainium2 kernel for 2-layer GAT (nn_GAT_50577534878113).

Strategy (8 NeuronCores, SPMD):
  - Nodes padded to NP = NBLK*128; dst-sorted edges sharded by dst-block range:
    core k owns BPC = NBLK/8 blocks of 128 destination nodes.
  - Dense phases (x@W1 etc.) replicated per core in bf16 (cheap on PE); the
    per-node payload table [h | a_src] is written to a per-core DRAM table.
  - Edge phase per 128-edge chunk (dst-block local): one K=1 indirect-DMA
    gather of payload rows by src id; one-hot matrices built on-chip
    (iota vs dst_rel is_equal) route a_dst expansion and the scatter-add as
    TensorE matmuls accumulating in PSUM per dst block. Softmax is computed
    without max-subtraction (logits are O(10), fp32 exp is exact enough) so
    denominators are aggregated alongside messages in the same matmuls.
  - Layer-2 local dense from the (transposed) layer-1 block outputs, then one
    AllGather distributes the global layer-2 payload table; the layer-2 edge
    phase mirrors layer 1. Output is node-sharded, host concatenates.

To keep per-core programs identical (SPMD), each core's node table is block-
rotated so its own 49 dst blocks come first; L1 gather indices are rotated to
match. The AllGather (in core order) restores the global node order for L2.
"""

import numpy as np
import ml_dtypes

bf16 = ml_dtypes.bfloat16

# Problem shapes (hardcoded per contract)
N_NODES = 50000
N_EDGES = 800000
IN_CH = 128
HEADS = 4
HIDDEN = 32
OUT_CH = 40
NEG = 0.2
NCORES = 8
BLK = 128

F1 = IN_CH + HEADS          # 132: [h1 (128) | a_src1 (4)]
F1T = F1 + HEADS            # 136: + a_dst1 (4)
F2 = OUT_CH + 2             # 42:  [h2 (40) | a_src2 | a_dst2]


def _build(NP, NBLK, BPC, CPB):
    import concourse.bass as bass
    import concourse.bacc as bacc
    import concourse.mybir as mybir
    import concourse.tile as tile

    dt = mybir.dt
    AL = mybir.AluOpType
    AF = mybir.ActivationFunctionType

    nc = bacc.Bacc("TRN2", target_bir_lowering=False, debug=False,
                   num_devices=NCORES)

    XT = nc.dram_tensor("xt", [128, NP], dt.bfloat16, kind="ExternalInput").ap()
    W1A = nc.dram_tensor("w1a", [128, F1T], dt.bfloat16, kind="ExternalInput").ap()
    W2A = nc.dram_tensor("w2a", [128, F2], dt.bfloat16, kind="ExternalInput").ap()
    IOTA = nc.dram_tensor("iota", [128, 128], dt.bfloat16, kind="ExternalInput").ap()
    IDB = nc.dram_tensor("idb", [128, 128], dt.bfloat16, kind="ExternalInput").ap()
    IDF = nc.dram_tensor("idf", [128, 128], dt.float32, kind="ExternalInput").ap()
    HSEL = nc.dram_tensor("hsel", [HEADS, 128], dt.bfloat16, kind="ExternalInput").ap()
    ONES1 = nc.dram_tensor("ones1", [1, OUT_CH], dt.bfloat16, kind="ExternalInput").ap()
    SRC1 = nc.dram_tensor("src1", [BPC, 128, CPB], dt.int32, kind="ExternalInput").ap()
    SRC2 = nc.dram_tensor("src2", [BPC, 128, CPB], dt.int32, kind="ExternalInput").ap()
    DREL = nc.dram_tensor("drel", [BPC, 128, CPB], dt.bfloat16, kind="ExternalInput").ap()
    OUT = nc.dram_tensor("out", [BPC * 128, OUT_CH], dt.float32, kind="ExternalOutput").ap()

    PL1 = nc.dram_tensor("pl1", [NP, F1], dt.bfloat16).ap()
    L2L = nc.dram_tensor("l2l", [BPC * 128, F2], dt.bfloat16).ap()
    PL2 = nc.dram_tensor("pl2", [NP, F2], dt.bfloat16, addr_space="Shared").ap()

    with tile.TileContext(nc) as tc:
        with tc.tile_pool(name="const", bufs=1) as cp, \
             tc.tile_pool(name="sb", bufs=3) as sp, \
             tc.tile_pool(name="blk", bufs=2) as bp, \
             tc.tile_pool(name="ps", bufs=2, space="PSUM") as pp:

            iota = cp.tile([128, 128], dt.bfloat16)
            nc.sync.dma_start(out=iota[:], in_=IOTA[:])
            idb = cp.tile([128, 128], dt.bfloat16)
            nc.sync.dma_start(out=idb[:], in_=IDB[:])
            idf = cp.tile([128, 128], dt.float32)
            nc.sync.dma_start(out=idf[:], in_=IDF[:])
            hsel = cp.tile([HEADS, 128], dt.bfloat16)
            nc.sync.dma_start(out=hsel[:], in_=HSEL[:])
            ones1 = cp.tile([1, OUT_CH], dt.bfloat16)
            nc.sync.dma_start(out=ones1[:], in_=ONES1[:])
            w1a = cp.tile([128, F1T], dt.bfloat16)
            nc.sync.dma_start(out=w1a[:], in_=W1A[:])
            w2a = cp.tile([128, F2], dt.bfloat16)
            nc.sync.dma_start(out=w2a[:], in_=W2A[:])

            adst1 = cp.tile([128, BPC * HEADS], dt.bfloat16)
            adst2 = cp.tile([128, BPC], dt.bfloat16)
            elu1t = cp.tile([128, BPC * 128], dt.bfloat16)

            # ---- stage 1: dense h1/a_src1/a_dst1 for ALL nodes; write PL1 ----
            for nb in range(NBLK):
                xt_t = sp.tile([128, 128], dt.bfloat16, tag="xt")
                nc.sync.dma_start(out=xt_t[:], in_=XT[:, nb * 128:(nb + 1) * 128])
                dps = pp.tile([128, F1T], dt.float32, tag="agg")
                nc.tensor.matmul(dps[:], lhsT=xt_t[:], rhs=w1a[:], start=True, stop=True)
                sb1 = sp.tile([128, F1], dt.bfloat16, tag="pl1sb")
                nc.scalar.copy(out=sb1[:], in_=dps[:, 0:F1])
                nc.sync.dma_start(out=PL1[nb * 128:(nb + 1) * 128, :], in_=sb1[:])
                if nb < BPC:
                    nc.vector.tensor_copy(adst1[:, nb * HEADS:(nb + 1) * HEADS],
                                          dps[:, F1:F1T])

            # ---- stage 2: layer-1 edge pass over own dst blocks ----
            for b in range(BPC):
                idx_t = sp.tile([128, CPB], dt.int32, tag="idx")
                nc.sync.dma_start(out=idx_t[:], in_=SRC1[b])
                dr_t = sp.tile([128, CPB], dt.bfloat16, tag="dr")
                nc.sync.dma_start(out=dr_t[:], in_=DREL[b])

                g = bp.tile([128, CPB * F1], dt.bfloat16, tag="g")
                for c in range(CPB):
                    nc.gpsimd.indirect_dma_start(
                        out=g[:, c * F1:(c + 1) * F1], out_offset=None,
                        in_=PL1[:],
                        in_offset=bass.IndirectOffsetOnAxis(ap=idx_t[:, c:c + 1], axis=0))

                # one-hot P[e, c, j] = (dst_rel[e,c] == j), all chunks at once
                P = bp.tile([128, CPB * 128], dt.bfloat16, tag="P")
                nc.vector.tensor_tensor(
                    out=P[:].rearrange("p (c j) -> p c j", j=128),
                    in0=dr_t[:, :, None].broadcast_to([128, CPB, 128]),
                    in1=iota[:, None, :].broadcast_to([128, CPB, 128]),
                    op=AL.is_equal)

                ADG = pp.tile([128, CPB * HEADS], dt.float32, tag="adg")
                AGG = pp.tile([128, 128], dt.float32, tag="agg")
                DEN = pp.tile([HEADS, 128], dt.float32, tag="den")

                for c in range(CPB):
                    trq = pp.tile([128, 128], dt.bfloat16, tag="trq")
                    nc.tensor.transpose(out=trq[:], in_=P[:, c * 128:(c + 1) * 128],
                                        identity=idb[:])
                    qd = sp.tile([128, 128], dt.bfloat16, tag="qd")
                    nc.scalar.copy(out=qd[:], in_=trq[:])
                    nc.tensor.matmul(ADG[:, c * HEADS:(c + 1) * HEADS], lhsT=qd[:],
                                     rhs=adst1[:, b * HEADS:(b + 1) * HEADS],
                                     start=True, stop=True)

                # logits, leaky-relu, exp (batched over the block's chunks)
                elog = sp.tile([128, CPB * HEADS], dt.float32, tag="elog")
                nc.vector.tensor_tensor(
                    out=elog[:].rearrange("p (c f) -> p c f", f=HEADS),
                    in0=ADG[:].rearrange("p (c f) -> p c f", f=HEADS),
                    in1=g[:].rearrange("p (c f) -> p c f", f=F1)[:, :, IN_CH:IN_CH + HEADS],
                    op=AL.add)
                lr = sp.tile([128, CPB * HEADS], dt.float32, tag="lr")
                nc.vector.tensor_scalar(out=lr[:], in0=elog[:], scalar1=NEG,
                                        scalar2=None, op0=AL.mult)
                nc.vector.tensor_tensor(out=lr[:], in0=lr[:], in1=elog[:], op=AL.max)
                s_all = sp.tile([128, CPB * HEADS], dt.bfloat16, tag="sall")
                nc.scalar.activation(out=s_all[:], in_=lr[:], func=AF.Exp)

                for c in range(CPB):
                    gs = sp.tile([128, 128], dt.bfloat16, tag="gs")
                    nc.vector.tensor_tensor(
                        out=gs[:].rearrange("p (h w) -> p h w", w=HIDDEN),
                        in0=g[:, c * F1:c * F1 + IN_CH].rearrange("p (h w) -> p h w", w=HIDDEN),
                        in1=s_all[:, c * HEADS:(c + 1) * HEADS][:, :, None]
                            .broadcast_to([128, HEADS, HIDDEN]),
                        op=AL.mult)
                    nc.tensor.matmul(AGG[:], lhsT=gs[:], rhs=P[:, c * 128:(c + 1) * 128],
                                     start=(c == 0), stop=(c == CPB - 1))
                    nc.tensor.matmul(DEN[:], lhsT=s_all[:, c * HEADS:(c + 1) * HEADS],
                                     rhs=P[:, c * 128:(c + 1) * 128],
                                     start=(c == 0), stop=(c == CPB - 1))

                # normalize + ELU, store transposed block to elu1t
                den_sb = sp.tile([HEADS, 128], dt.float32, tag="densb")
                nc.vector.tensor_scalar(out=den_sb[:], in0=DEN[:], scalar1=1e-16,
                                        scalar2=None, op0=AL.add)
                rec = sp.tile([HEADS, 128], dt.float32, tag="rec")
                nc.vector.reciprocal(out=rec[:], in_=den_sb[:])
                rec_bf = sp.tile([HEADS, 128], dt.bfloat16, tag="recbf")
                nc.vector.tensor_copy(rec_bf[:], rec[:])
                rep = pp.tile([128, 128], dt.float32, tag="trq")
                nc.tensor.matmul(rep[:], lhsT=hsel[:], rhs=rec_bf[:], start=True, stop=True)
                rep_sb = sp.tile([128, 128], dt.float32, tag="repsb")
                nc.scalar.copy(out=rep_sb[:], in_=rep[:])
                t1 = sp.tile([128, 128], dt.float32, tag="t1")
                nc.vector.tensor_tensor(out=t1[:], in0=AGG[:], in1=rep_sb[:], op=AL.mult)
                m1 = sp.tile([128, 128], dt.float32, tag="m1")
                nc.vector.tensor_scalar(out=m1[:], in0=t1[:], scalar1=0.0,
                                        scalar2=None, op0=AL.min)
                u1 = sp.tile([128, 128], dt.float32, tag="u1")
                nc.scalar.activation(out=u1[:], in_=m1[:], func=AF.Exp)
                pm1 = sp.tile([128, 128], dt.float32, tag="pm1")
                nc.vector.tensor_scalar(out=pm1[:], in0=t1[:], scalar1=0.0, scalar2=-1.0,
                                        op0=AL.max, op1=AL.add)
                nc.vector.tensor_tensor(out=elu1t[:, b * 128:(b + 1) * 128],
                                        in0=pm1[:], in1=u1[:], op=AL.add)

            # ---- stage 3: layer-2 local dense + AllGather ----
            for b in range(BPC):
                d2 = pp.tile([128, F2], dt.float32, tag="agg")
                nc.tensor.matmul(d2[:], lhsT=elu1t[:, b * 128:(b + 1) * 128],
                                 rhs=w2a[:], start=True, stop=True)
                sb2 = sp.tile([128, F2], dt.bfloat16, tag="sb2")
                nc.scalar.copy(out=sb2[:], in_=d2[:])
                nc.sync.dma_start(out=L2L[b * 128:(b + 1) * 128, :], in_=sb2[:])
                nc.vector.tensor_copy(adst2[:, b:b + 1], d2[:, F2 - 1:F2])

            nc.gpsimd.collective_compute(
                "AllGather", mybir.AluOpType.bypass,
                replica_groups=[list(range(NCORES))],
                ins=[L2L[:, :]], outs=[PL2[:, :]])

            # ---- stage 4: layer-2 edge pass ----
            for b in range(BPC):
                idx2 = sp.tile([128, CPB], dt.int32, tag="idx")
                nc.sync.dma_start(out=idx2[:], in_=SRC2[b])
                dr_t = sp.tile([128, CPB], dt.bfloat16, tag="dr")
                nc.sync.dma_start(out=dr_t[:], in_=DREL[b])

                g2 = bp.tile([128, CPB * F2], dt.bfloat16, tag="g2")
                for c in range(CPB):
                    nc.gpsimd.indirect_dma_start(
                        out=g2[:, c * F2:(c + 1) * F2], out_offset=None,
                        in_=PL2[:],
                        in_offset=bass.IndirectOffsetOnAxis(ap=idx2[:, c:c + 1], axis=0))

                P = bp.tile([128, CPB * 128], dt.bfloat16, tag="P")
                nc.vector.tensor_tensor(
                    out=P[:].rearrange("p (c j) -> p c j", j=128),
                    in0=dr_t[:, :, None].broadcast_to([128, CPB, 128]),
                    in1=iota[:, None, :].broadcast_to([128, CPB, 128]),
                    op=AL.is_equal)

                ADG2 = pp.tile([128, CPB], dt.float32, tag="adg")
                AGG2 = pp.tile([OUT_CH, 128], dt.float32, tag="agg")

                for c in range(CPB):
                    trq = pp.tile([128, 128], dt.bfloat16, tag="trq")
                    nc.tensor.transpose(out=trq[:], in_=P[:, c * 128:(c + 1) * 128],
                                        identity=idb[:])
                    qd = sp.tile([128, 128], dt.bfloat16, tag="qd")
                    nc.scalar.copy(out=qd[:], in_=trq[:])
                    nc.tensor.matmul(ADG2[:, c:c + 1], lhsT=qd[:],
                                     rhs=adst2[:, b:b + 1], start=True, stop=True)

                elog2 = sp.tile([128, CPB], dt.float32, tag="elog")
                nc.vector.tensor_tensor(
                    out=elog2[:, :, None],
                    in0=ADG2[:, :, None],
                    in1=g2[:].rearrange("p (c f) -> p c f", f=F2)[:, :, OUT_CH:OUT_CH + 1],
                    op=AL.add)
                lr2 = sp.tile([128, CPB], dt.float32, tag="lr")
                nc.vector.tensor_scalar(out=lr2[:], in0=elog2[:], scalar1=NEG,
                                        scalar2=None, op0=AL.mult)
                nc.vector.tensor_tensor(out=lr2[:], in0=lr2[:], in1=elog2[:], op=AL.max)
                s2 = sp.tile([128, CPB], dt.float32, tag="sall2")
                nc.scalar.activation(out=s2[:], in_=lr2[:], func=AF.Exp)
                s2b = sp.tile([128, CPB], dt.bfloat16, tag="sall")
                nc.scalar.copy(out=s2b[:], in_=s2[:])

                DEN2 = pp.tile([1, 128], dt.float32, tag="den")
                for c in range(CPB):
                    gs2 = sp.tile([128, OUT_CH], dt.bfloat16, tag="gs")
                    nc.vector.tensor_scalar(out=gs2[:],
                                            in0=g2[:, c * F2:c * F2 + OUT_CH],
                                            scalar1=s2[:, c:c + 1], scalar2=None,
                                            op0=AL.mult)
                    nc.tensor.matmul(AGG2[:], lhsT=gs2[:], rhs=P[:, c * 128:(c + 1) * 128],
                                     start=(c == 0), stop=(c == CPB - 1))
                    nc.tensor.matmul(DEN2[:], lhsT=s2b[:, c:c + 1],
                                     rhs=P[:, c * 128:(c + 1) * 128],
                                     start=(c == 0), stop=(c == CPB - 1))

                ag2sb = sp.tile([OUT_CH, 128], dt.float32, tag="ag2sb")
                nc.scalar.copy(out=ag2sb[:], in_=AGG2[:])
                den2 = sp.tile([1, 128], dt.float32, tag="densb")
                nc.vector.tensor_scalar(out=den2[:], in0=DEN2[:],
                                        scalar1=1e-16, scalar2=None, op0=AL.add)
                rec2 = sp.tile([1, 128], dt.float32, tag="rec")
                nc.vector.reciprocal(out=rec2[:], in_=den2[:])
                rec2bf = sp.tile([1, 128], dt.bfloat16, tag="recbf")
                nc.vector.tensor_copy(rec2bf[:], rec2[:])
                rep2 = pp.tile([OUT_CH, 128], dt.float32, tag="trq")
                nc.tensor.matmul(rep2[:], lhsT=ones1[:], rhs=rec2bf[:], start=True, stop=True)
                rep2sb = sp.tile([OUT_CH, 128], dt.float32, tag="repsb")
                nc.scalar.copy(out=rep2sb[:], in_=rep2[:])
                o2 = sp.tile([OUT_CH, 128], dt.float32, tag="t1")
                nc.vector.tensor_tensor(out=o2[:], in0=ag2sb[0:OUT_CH, :],
                                        in1=rep2sb[:], op=AL.mult)
                otp = pp.tile([128, OUT_CH], dt.float32, tag="den")
                nc.tensor.transpose(out=otp[:], in_=o2[:],
                                    identity=idf[0:OUT_CH, 0:OUT_CH])
                osb = sp.tile([128, OUT_CH], dt.float32, tag="osb")
                nc.scalar.copy(out=osb[:], in_=otp[:])
                nc.sync.dma_start(out=OUT[b * 128:(b + 1) * 128, :], in_=osb[:])

    nc.compile()
    return nc


def _host_prep(x, edge_index, W1, att_src1, att_dst1, W2, att_src2, att_dst2,
               n_nodes, n_edges):
    NBLK = -(-n_nodes // BLK)
    NBLK = -(-NBLK // NCORES) * NCORES
    NP = NBLK * BLK
    BPC = NBLK // NCORES

    x = np.asarray(x, np.float32)
    W1 = np.asarray(W1, np.float32)
    W2 = np.asarray(W2, np.float32)
    att_src1 = np.asarray(att_src1, np.float32)
    att_dst1 = np.asarray(att_dst1, np.float32)
    att_src2 = np.asarray(att_src2, np.float32)
    att_dst2 = np.asarray(att_dst2, np.float32)
    H, C = att_src1.shape

    xp = np.zeros((NP, IN_CH), np.float32)
    xp[:n_nodes] = x
    XT = np.ascontiguousarray(xp.T).astype(bf16)          # [128, NP]

    Asrc1 = np.zeros((H * C, H), np.float32)
    Adst1 = np.zeros((H * C, H), np.float32)
    for h in range(H):
        Asrc1[h * C:(h + 1) * C, h] = att_src1[h]
        Adst1[h * C:(h + 1) * C, h] = att_dst1[h]
    W1A = np.concatenate([W1, W1 @ Asrc1, W1 @ Adst1], axis=1).astype(bf16)  # [128,136]
    W2A = np.concatenate([W2, W2 @ att_src2.T, W2 @ att_dst2.T], axis=1).astype(bf16)

    IOTA = np.broadcast_to(np.arange(128, dtype=np.float32), (128, 128)).astype(bf16)
    IDB = np.eye(128, dtype=np.float32).astype(bf16)
    IDF = np.eye(128, dtype=np.float32)
    HSEL = np.zeros((H, 128), np.float32)
    for h in range(H):
        HSEL[h, h * C:(h + 1) * C] = 1.0
    HSEL = HSEL.astype(bf16)
    ONES1 = np.ones((1, OUT_CH), np.float32).astype(bf16)

    src = np.asarray(edge_index[0], np.int64)
    dst = np.asarray(edge_index[1], np.int64)
    order = np.argsort(dst, kind="stable")
    ss = src[order]
    dd = dst[order]
    blk = dd // BLK
    bstart = np.searchsorted(blk, np.arange(NBLK))
    bcount = np.diff(np.append(bstart, n_edges))
    CPB = max(1, int(-(-bcount.max() // 128)))

    rank = np.arange(n_edges) - bstart[blk]
    core = blk // BPC
    b_in_core = blk % BPC
    chunk = rank // 128
    lane = rank % 128

    SRC1 = np.zeros((NCORES, BPC, 128, CPB), np.int32)
    SRC2 = np.zeros((NCORES, BPC, 128, CPB), np.int32)
    DREL = np.full((NCORES, BPC, 128, CPB), 200.0, np.float32)
    XTs = []
    for k in range(NCORES):
        sel = core == k
        bb, pp_, cc = b_in_core[sel], lane[sel], chunk[sel]
        s_k = ss[sel]
        rot = ((s_k // BLK - k * BPC) % NBLK) * BLK + (s_k % BLK)
        SRC1[k][bb, pp_, cc] = rot
        SRC2[k][bb, pp_, cc] = s_k
        DREL[k][bb, pp_, cc] = (dd[sel] % BLK).astype(np.float32)
        XTb = XT.reshape(128, NBLK, BLK)
        XTs.append(np.ascontiguousarray(
            np.roll(XTb, -k * BPC, axis=1).reshape(128, NP)))
    DREL = DREL.astype(bf16)

    consts = dict(w1a=W1A, w2a=W2A, iota=IOTA, idb=IDB, idf=IDF,
                  hsel=HSEL, ones1=ONES1)
    in_maps = []
    for k in range(NCORES):
        m = dict(consts)
        m["xt"] = XTs[k]
        m["src1"] = SRC1[k]
        m["src2"] = SRC2[k]
        m["drel"] = DREL[k]
        in_maps.append(m)
    return NP, NBLK, BPC, CPB, in_maps


_CACHE = {}


def _run(x, edge_index, W1, att_src1, att_dst1, W2, att_src2, att_dst2,
         n_nodes, n_edges, trace=False):
    from concourse import bass_utils
    NP, NBLK, BPC, CPB, in_maps = _host_prep(
        x, edge_index, W1, att_src1, att_dst1, W2, att_src2, att_dst2,
        n_nodes, n_edges)
    key = (NP, CPB)
    if key not in _CACHE:
        _CACHE[key] = _build(NP, NBLK, BPC, CPB)
    nc = _CACHE[key]
    res = bass_utils.run_bass_kernel_spmd(nc, in_maps, core_ids=list(range(NCORES)),
                                          trace=trace)
    out = np.concatenate([np.asarray(res.results[k]["out"]) for k in range(NCORES)],
                         axis=0)[:n_nodes]
    return np.ascontiguousarray(out.astype(np.float32)), res


def kernel(x, edge_index, W1, att_src1, att_dst1, W2, att_src2, att_dst2):
    out, _ = _run(x, edge_index, W1, att_src1, att_dst1, W2, att_src2, att_dst2,
                  N_NODES, N_EDGES)
    return out


# revision 8
# speedup vs baseline: 1.0779x; 1.0779x over previous
"""Bass/Trainium2 kernel for 2-layer GAT (nn_GAT_50577534878113).

Strategy (8 NeuronCores, SPMD):
  - Nodes padded to NP = NBLK*128; dst-sorted edges sharded by dst-block range:
    core k owns BPC = NBLK/8 blocks of 128 destination nodes.
  - Dense phases (x@W1 etc.) replicated per core in bf16 (cheap on PE); the
    per-node payload table [h | a_src] is written to a per-core DRAM table.
  - Edge phase per 128-edge chunk (dst-block local): one K=1 indirect-DMA
    gather of payload rows by src id; one-hot matrices built on-chip
    (iota vs dst_rel is_equal) route a_dst expansion and the scatter-add as
    TensorE matmuls accumulating in PSUM per dst block. Softmax is computed
    without max-subtraction (logits are O(10), fp32 exp is exact enough) so
    denominators are aggregated alongside messages in the same matmuls.
  - Layer-2 local dense from the (transposed) layer-1 block outputs, then one
    AllGather distributes the global layer-2 payload table; the layer-2 edge
    phase mirrors layer 1. Output is node-sharded, host concatenates.

To keep per-core programs identical (SPMD), each core's node table is block-
rotated so its own 49 dst blocks come first; L1 gather indices are rotated to
match. The AllGather (in core order) restores the global node order for L2.
"""

import numpy as np
import ml_dtypes

bf16 = ml_dtypes.bfloat16

# Problem shapes (hardcoded per contract)
N_NODES = 50000
N_EDGES = 800000
IN_CH = 128
HEADS = 4
HIDDEN = 32
OUT_CH = 40
NEG = 0.2
NCORES = 8
BLK = 128

F1 = IN_CH + HEADS          # 132: [h1 (128) | a_src1 (4)]
F1T = F1 + HEADS            # 136: + a_dst1 (4)
F2 = OUT_CH + 2             # 42:  [h2 (40) | a_src2 | a_dst2]


def _build(NP, NBLK, BPC, CPB):
    import concourse.bass as bass
    import concourse.bacc as bacc
    import concourse.mybir as mybir
    import concourse.tile as tile

    dt = mybir.dt
    AL = mybir.AluOpType
    AF = mybir.ActivationFunctionType

    nc = bacc.Bacc("TRN2", target_bir_lowering=False, debug=False,
                   num_devices=NCORES)

    XT = nc.dram_tensor("xt", [128, NP], dt.bfloat16, kind="ExternalInput").ap()
    W1A = nc.dram_tensor("w1a", [128, F1T], dt.bfloat16, kind="ExternalInput").ap()
    W2A = nc.dram_tensor("w2a", [128, F2], dt.bfloat16, kind="ExternalInput").ap()
    IOTA = nc.dram_tensor("iota", [128, 128], dt.bfloat16, kind="ExternalInput").ap()
    IDB = nc.dram_tensor("idb", [128, 128], dt.bfloat16, kind="ExternalInput").ap()
    IDF = nc.dram_tensor("idf", [128, 128], dt.float32, kind="ExternalInput").ap()
    HSEL = nc.dram_tensor("hsel", [HEADS, 128], dt.bfloat16, kind="ExternalInput").ap()
    ONES1 = nc.dram_tensor("ones1", [1, OUT_CH], dt.bfloat16, kind="ExternalInput").ap()
    SRC1 = nc.dram_tensor("src1", [BPC, 128, CPB], dt.int32, kind="ExternalInput").ap()
    SRC2 = nc.dram_tensor("src2", [BPC, 128, CPB], dt.int32, kind="ExternalInput").ap()
    DREL = nc.dram_tensor("drel", [BPC, 128, CPB], dt.bfloat16, kind="ExternalInput").ap()
    OUT = nc.dram_tensor("out", [BPC * 128, OUT_CH], dt.float32, kind="ExternalOutput").ap()

    PL1 = nc.dram_tensor("pl1", [NP, F1], dt.bfloat16).ap()
    L2L = nc.dram_tensor("l2l", [BPC * 128, F2], dt.bfloat16).ap()
    PL2 = nc.dram_tensor("pl2", [NP, F2], dt.bfloat16, addr_space="Shared").ap()

    with tile.TileContext(nc) as tc:
        with tc.tile_pool(name="const", bufs=1) as cp, \
             tc.tile_pool(name="sb", bufs=3) as sp, \
             tc.tile_pool(name="blk", bufs=3) as bp, \
             tc.tile_pool(name="ps", bufs=2, space="PSUM") as pp:

            iota = cp.tile([128, 128], dt.bfloat16)
            nc.sync.dma_start(out=iota[:], in_=IOTA[:])
            idb = cp.tile([128, 128], dt.bfloat16)
            nc.sync.dma_start(out=idb[:], in_=IDB[:])
            idf = cp.tile([128, 128], dt.float32)
            nc.sync.dma_start(out=idf[:], in_=IDF[:])
            hsel = cp.tile([HEADS, 128], dt.bfloat16)
            nc.sync.dma_start(out=hsel[:], in_=HSEL[:])
            ones1 = cp.tile([1, OUT_CH], dt.bfloat16)
            nc.sync.dma_start(out=ones1[:], in_=ONES1[:])
            w1a = cp.tile([128, F1T], dt.bfloat16)
            nc.sync.dma_start(out=w1a[:], in_=W1A[:])
            w2a = cp.tile([128, F2], dt.bfloat16)
            nc.sync.dma_start(out=w2a[:], in_=W2A[:])

            adst1 = cp.tile([128, BPC * HEADS], dt.bfloat16)
            adst2 = cp.tile([128, BPC], dt.bfloat16)
            elu1t = cp.tile([128, BPC * 128], dt.bfloat16)

            # ---- stage 1: dense h1/a_src1/a_dst1 for ALL nodes; write PL1 ----
            XCH = 49 if NBLK % 49 == 0 else 1   # blocks per big x load
            for g0 in range(0, NBLK, XCH):
                xt_big = sp.tile([128, XCH * 128], dt.bfloat16, tag="xt")
                nc.sync.dma_start(out=xt_big[:],
                                  in_=XT[:, g0 * 128:(g0 + XCH) * 128])
                for j in range(XCH):
                    nb = g0 + j
                    dps = pp.tile([128, F1T], dt.float32,
                                  tag=["agg", "trq", "den", "adg"][nb % 4])
                    nc.tensor.matmul(dps[:], lhsT=xt_big[:, j * 128:(j + 1) * 128],
                                     rhs=w1a[:], start=True, stop=True)
                    sb1 = sp.tile([128, F1], dt.bfloat16, tag=f"pl1sb{nb % 2}")
                    nc.vector.tensor_copy(sb1[:], dps[:, 0:F1])
                    eng = nc.scalar if nb % 2 else nc.sync
                    eng.dma_start(out=PL1[nb * 128:(nb + 1) * 128, :], in_=sb1[:])
                    if nb < BPC:
                        nc.vector.tensor_copy(adst1[:, nb * HEADS:(nb + 1) * HEADS],
                                              dps[:, F1:F1T])

            # ---- stage 2: layer-1 edge pass over own dst blocks ----
            for b in range(BPC):
                idx_t = sp.tile([128, CPB], dt.int32, tag="idx")
                nc.sync.dma_start(out=idx_t[:], in_=SRC1[b])
                dr_t = sp.tile([128, CPB], dt.bfloat16, tag="dr")
                nc.sync.dma_start(out=dr_t[:], in_=DREL[b])

                g = bp.tile([128, CPB * F1], dt.bfloat16, tag="g")
                for c in range(CPB):
                    nc.gpsimd.indirect_dma_start(
                        out=g[:, c * F1:(c + 1) * F1], out_offset=None,
                        in_=PL1[:],
                        in_offset=bass.IndirectOffsetOnAxis(ap=idx_t[:, c:c + 1], axis=0))

                # one-hot P[e, c, j] = (dst_rel[e,c] == j), all chunks at once
                P = bp.tile([128, CPB * 128], dt.bfloat16, tag="P")
                nc.vector.tensor_tensor(
                    out=P[:].rearrange("p (c j) -> p c j", j=128),
                    in0=dr_t[:, :, None].broadcast_to([128, CPB, 128]),
                    in1=iota[:, None, :].broadcast_to([128, CPB, 128]),
                    op=AL.is_equal)

                ADG = pp.tile([128, CPB * HEADS], dt.float32, tag="adg")
                AGG = pp.tile([128, 128], dt.float32, tag="agg")
                DEN = pp.tile([HEADS, 128], dt.float32, tag="den")

                for c in range(CPB):
                    trq = pp.tile([128, 128], dt.bfloat16, tag="trq")
                    nc.tensor.transpose(out=trq[:], in_=P[:, c * 128:(c + 1) * 128],
                                        identity=idb[:])
                    qd = sp.tile([128, 128], dt.bfloat16, tag="qd")
                    nc.scalar.copy(out=qd[:], in_=trq[:])
                    nc.tensor.matmul(ADG[:, c * HEADS:(c + 1) * HEADS], lhsT=qd[:],
                                     rhs=adst1[:, b * HEADS:(b + 1) * HEADS],
                                     start=True, stop=True)

                # logits, leaky-relu, exp (batched over the block's chunks)
                elog = sp.tile([128, CPB * HEADS], dt.float32, tag="elog")
                nc.vector.tensor_tensor(
                    out=elog[:].rearrange("p (c f) -> p c f", f=HEADS),
                    in0=ADG[:].rearrange("p (c f) -> p c f", f=HEADS),
                    in1=g[:].rearrange("p (c f) -> p c f", f=F1)[:, :, IN_CH:IN_CH + HEADS],
                    op=AL.add)
                lr = sp.tile([128, CPB * HEADS], dt.float32, tag="lr")
                nc.vector.tensor_scalar(out=lr[:], in0=elog[:], scalar1=NEG,
                                        scalar2=None, op0=AL.mult)
                nc.vector.tensor_tensor(out=lr[:], in0=lr[:], in1=elog[:], op=AL.max)
                s_all = sp.tile([128, CPB * HEADS], dt.bfloat16, tag="sall")
                nc.scalar.activation(out=s_all[:], in_=lr[:], func=AF.Exp)

                for c in range(CPB):
                    gs = sp.tile([128, 128], dt.bfloat16, tag="gs")
                    nc.vector.tensor_tensor(
                        out=gs[:].rearrange("p (h w) -> p h w", w=HIDDEN),
                        in0=g[:, c * F1:c * F1 + IN_CH].rearrange("p (h w) -> p h w", w=HIDDEN),
                        in1=s_all[:, c * HEADS:(c + 1) * HEADS][:, :, None]
                            .broadcast_to([128, HEADS, HIDDEN]),
                        op=AL.mult)
                    nc.tensor.matmul(AGG[:], lhsT=gs[:], rhs=P[:, c * 128:(c + 1) * 128],
                                     start=(c == 0), stop=(c == CPB - 1))
                    nc.tensor.matmul(DEN[:], lhsT=s_all[:, c * HEADS:(c + 1) * HEADS],
                                     rhs=P[:, c * 128:(c + 1) * 128],
                                     start=(c == 0), stop=(c == CPB - 1))

                # normalize + ELU, store transposed block to elu1t
                den_sb = sp.tile([HEADS, 128], dt.float32, tag="densb")
                nc.vector.tensor_scalar(out=den_sb[:], in0=DEN[:], scalar1=1e-16,
                                        scalar2=None, op0=AL.add)
                rec = sp.tile([HEADS, 128], dt.float32, tag="rec")
                nc.vector.reciprocal_approx_fast(out=rec[:], in_=den_sb[:])
                rec_bf = sp.tile([HEADS, 128], dt.bfloat16, tag="recbf")
                nc.vector.tensor_copy(rec_bf[:], rec[:])
                rep = pp.tile([128, 128], dt.float32, tag="trq")
                nc.tensor.matmul(rep[:], lhsT=hsel[:], rhs=rec_bf[:], start=True, stop=True)
                rep_sb = sp.tile([128, 128], dt.float32, tag="repsb")
                nc.scalar.copy(out=rep_sb[:], in_=rep[:])
                t1 = sp.tile([128, 128], dt.float32, tag="t1")
                nc.vector.tensor_tensor(out=t1[:], in0=AGG[:], in1=rep_sb[:], op=AL.mult)
                m1 = sp.tile([128, 128], dt.float32, tag="m1")
                nc.vector.tensor_scalar(out=m1[:], in0=t1[:], scalar1=0.0,
                                        scalar2=None, op0=AL.min)
                u1 = sp.tile([128, 128], dt.float32, tag="u1")
                nc.scalar.activation(out=u1[:], in_=m1[:], func=AF.Exp)
                pm1 = sp.tile([128, 128], dt.float32, tag="pm1")
                nc.vector.tensor_scalar(out=pm1[:], in0=t1[:], scalar1=0.0, scalar2=-1.0,
                                        op0=AL.max, op1=AL.add)
                nc.vector.tensor_tensor(out=elu1t[:, b * 128:(b + 1) * 128],
                                        in0=pm1[:], in1=u1[:], op=AL.add)

                # layer-2 local dense for this block (fused stage 3)
                d2 = pp.tile([128, F2], dt.float32, tag="adg")
                nc.tensor.matmul(d2[:], lhsT=elu1t[:, b * 128:(b + 1) * 128],
                                 rhs=w2a[:], start=True, stop=True)
                sb2 = sp.tile([128, F2], dt.bfloat16, tag="sb2")
                nc.scalar.copy(out=sb2[:], in_=d2[:])
                nc.scalar.dma_start(out=L2L[b * 128:(b + 1) * 128, :], in_=sb2[:])
                nc.vector.tensor_copy(adst2[:, b:b + 1], d2[:, F2 - 1:F2])

            nc.gpsimd.collective_compute(
                "AllGather", mybir.AluOpType.bypass,
                replica_groups=[list(range(NCORES))],
                ins=[L2L[:, :]], outs=[PL2[:, :]])

            # ---- stage 4: layer-2 edge pass ----
            for b in range(BPC):
                idx2 = sp.tile([128, CPB], dt.int32, tag="idx")
                nc.sync.dma_start(out=idx2[:], in_=SRC2[b])
                dr_t = sp.tile([128, CPB], dt.bfloat16, tag="dr")
                nc.sync.dma_start(out=dr_t[:], in_=DREL[b])

                g2 = bp.tile([128, CPB * F2], dt.bfloat16, tag="g2")
                for c in range(CPB):
                    nc.gpsimd.indirect_dma_start(
                        out=g2[:, c * F2:(c + 1) * F2], out_offset=None,
                        in_=PL2[:],
                        in_offset=bass.IndirectOffsetOnAxis(ap=idx2[:, c:c + 1], axis=0))

                P = bp.tile([128, CPB * 128], dt.bfloat16, tag="P")
                nc.vector.tensor_tensor(
                    out=P[:].rearrange("p (c j) -> p c j", j=128),
                    in0=dr_t[:, :, None].broadcast_to([128, CPB, 128]),
                    in1=iota[:, None, :].broadcast_to([128, CPB, 128]),
                    op=AL.is_equal)

                ADG2 = pp.tile([128, CPB], dt.float32, tag="adg")
                AGG2 = pp.tile([OUT_CH, 128], dt.float32, tag="agg")

                for c in range(CPB):
                    trq = pp.tile([128, 128], dt.bfloat16, tag="trq")
                    nc.tensor.transpose(out=trq[:], in_=P[:, c * 128:(c + 1) * 128],
                                        identity=idb[:])
                    qd = sp.tile([128, 128], dt.bfloat16, tag="qd")
                    nc.scalar.copy(out=qd[:], in_=trq[:])
                    nc.tensor.matmul(ADG2[:, c:c + 1], lhsT=qd[:],
                                     rhs=adst2[:, b:b + 1], start=True, stop=True)

                elog2 = sp.tile([128, CPB], dt.float32, tag="elog")
                nc.vector.tensor_tensor(
                    out=elog2[:, :, None],
                    in0=ADG2[:, :, None],
                    in1=g2[:].rearrange("p (c f) -> p c f", f=F2)[:, :, OUT_CH:OUT_CH + 1],
                    op=AL.add)
                lr2 = sp.tile([128, CPB], dt.float32, tag="lr")
                nc.vector.tensor_scalar(out=lr2[:], in0=elog2[:], scalar1=NEG,
                                        scalar2=None, op0=AL.mult)
                nc.vector.tensor_tensor(out=lr2[:], in0=lr2[:], in1=elog2[:], op=AL.max)
                s2 = sp.tile([128, CPB], dt.float32, tag="sall2")
                nc.scalar.activation(out=s2[:], in_=lr2[:], func=AF.Exp)
                s2b = sp.tile([128, CPB], dt.bfloat16, tag="sall")
                nc.scalar.copy(out=s2b[:], in_=s2[:])

                DEN2 = pp.tile([1, 128], dt.float32, tag="den")
                for c in range(CPB):
                    gs2 = sp.tile([128, OUT_CH], dt.bfloat16, tag="gs")
                    nc.vector.tensor_scalar(out=gs2[:],
                                            in0=g2[:, c * F2:c * F2 + OUT_CH],
                                            scalar1=s2[:, c:c + 1], scalar2=None,
                                            op0=AL.mult)
                    nc.tensor.matmul(AGG2[:], lhsT=gs2[:], rhs=P[:, c * 128:(c + 1) * 128],
                                     start=(c == 0), stop=(c == CPB - 1))
                    nc.tensor.matmul(DEN2[:], lhsT=s2b[:, c:c + 1],
                                     rhs=P[:, c * 128:(c + 1) * 128],
                                     start=(c == 0), stop=(c == CPB - 1))

                ag2sb = sp.tile([OUT_CH, 128], dt.float32, tag="ag2sb")
                nc.scalar.copy(out=ag2sb[:], in_=AGG2[:])
                den2 = sp.tile([1, 128], dt.float32, tag="densb")
                nc.vector.tensor_scalar(out=den2[:], in0=DEN2[:],
                                        scalar1=1e-16, scalar2=None, op0=AL.add)
                rec2 = sp.tile([1, 128], dt.float32, tag="rec")
                nc.vector.reciprocal_approx_fast(out=rec2[:], in_=den2[:])
                rec2bf = sp.tile([1, 128], dt.bfloat16, tag="recbf")
                nc.vector.tensor_copy(rec2bf[:], rec2[:])
                rep2 = pp.tile([OUT_CH, 128], dt.float32, tag="trq")
                nc.tensor.matmul(rep2[:], lhsT=ones1[:], rhs=rec2bf[:], start=True, stop=True)
                rep2sb = sp.tile([OUT_CH, 128], dt.float32, tag="repsb")
                nc.scalar.copy(out=rep2sb[:], in_=rep2[:])
                o2 = sp.tile([OUT_CH, 128], dt.float32, tag="t1")
                nc.vector.tensor_tensor(out=o2[:], in0=ag2sb[0:OUT_CH, :],
                                        in1=rep2sb[:], op=AL.mult)
                otp = pp.tile([128, OUT_CH], dt.float32, tag="den")
                nc.tensor.transpose(out=otp[:], in_=o2[:],
                                    identity=idf[0:OUT_CH, 0:OUT_CH])
                osb = sp.tile([128, OUT_CH], dt.float32, tag="osb")
                nc.scalar.copy(out=osb[:], in_=otp[:])
                nc.sync.dma_start(out=OUT[b * 128:(b + 1) * 128, :], in_=osb[:])

    nc.compile()
    return nc


def _host_prep(x, edge_index, W1, att_src1, att_dst1, W2, att_src2, att_dst2,
               n_nodes, n_edges):
    NBLK = -(-n_nodes // BLK)
    NBLK = -(-NBLK // NCORES) * NCORES
    NP = NBLK * BLK
    BPC = NBLK // NCORES

    x = np.asarray(x, np.float32)
    W1 = np.asarray(W1, np.float32)
    W2 = np.asarray(W2, np.float32)
    att_src1 = np.asarray(att_src1, np.float32)
    att_dst1 = np.asarray(att_dst1, np.float32)
    att_src2 = np.asarray(att_src2, np.float32)
    att_dst2 = np.asarray(att_dst2, np.float32)
    H, C = att_src1.shape

    xp = np.zeros((NP, IN_CH), np.float32)
    xp[:n_nodes] = x
    XT = np.ascontiguousarray(xp.T).astype(bf16)          # [128, NP]

    Asrc1 = np.zeros((H * C, H), np.float32)
    Adst1 = np.zeros((H * C, H), np.float32)
    for h in range(H):
        Asrc1[h * C:(h + 1) * C, h] = att_src1[h]
        Adst1[h * C:(h + 1) * C, h] = att_dst1[h]
    W1A = np.concatenate([W1, W1 @ Asrc1, W1 @ Adst1], axis=1).astype(bf16)  # [128,136]
    W2A = np.concatenate([W2, W2 @ att_src2.T, W2 @ att_dst2.T], axis=1).astype(bf16)

    IOTA = np.broadcast_to(np.arange(128, dtype=np.float32), (128, 128)).astype(bf16)
    IDB = np.eye(128, dtype=np.float32).astype(bf16)
    IDF = np.eye(128, dtype=np.float32)
    HSEL = np.zeros((H, 128), np.float32)
    for h in range(H):
        HSEL[h, h * C:(h + 1) * C] = 1.0
    HSEL = HSEL.astype(bf16)
    ONES1 = np.ones((1, OUT_CH), np.float32).astype(bf16)

    src = np.asarray(edge_index[0], np.int64)
    dst = np.asarray(edge_index[1], np.int64)
    order = np.argsort(dst, kind="stable")
    ss = src[order]
    dd = dst[order]
    blk = dd // BLK
    bstart = np.searchsorted(blk, np.arange(NBLK))
    bcount = np.diff(np.append(bstart, n_edges))
    CPB = max(1, int(-(-bcount.max() // 128)))

    rank = np.arange(n_edges) - bstart[blk]
    core = blk // BPC
    b_in_core = blk % BPC
    chunk = rank // 128
    lane = rank % 128

    SRC1 = np.zeros((NCORES, BPC, 128, CPB), np.int32)
    SRC2 = np.zeros((NCORES, BPC, 128, CPB), np.int32)
    DREL = np.full((NCORES, BPC, 128, CPB), 200.0, np.float32)
    XTs = []
    for k in range(NCORES):
        sel = core == k
        bb, pp_, cc = b_in_core[sel], lane[sel], chunk[sel]
        s_k = ss[sel]
        rot = ((s_k // BLK - k * BPC) % NBLK) * BLK + (s_k % BLK)
        SRC1[k][bb, pp_, cc] = rot
        SRC2[k][bb, pp_, cc] = s_k
        DREL[k][bb, pp_, cc] = (dd[sel] % BLK).astype(np.float32)
        XTb = XT.reshape(128, NBLK, BLK)
        XTs.append(np.ascontiguousarray(
            np.roll(XTb, -k * BPC, axis=1).reshape(128, NP)))
    DREL = DREL.astype(bf16)

    consts = dict(w1a=W1A, w2a=W2A, iota=IOTA, idb=IDB, idf=IDF,
                  hsel=HSEL, ones1=ONES1)
    in_maps = []
    for k in range(NCORES):
        m = dict(consts)
        m["xt"] = XTs[k]
        m["src1"] = SRC1[k]
        m["src2"] = SRC2[k]
        m["drel"] = DREL[k]
        in_maps.append(m)
    return NP, NBLK, BPC, CPB, in_maps


_CACHE = {}


def _run(x, edge_index, W1, att_src1, att_dst1, W2, att_src2, att_dst2,
         n_nodes, n_edges, trace=False):
    from concourse import bass_utils
    NP, NBLK, BPC, CPB, in_maps = _host_prep(
        x, edge_index, W1, att_src1, att_dst1, W2, att_src2, att_dst2,
        n_nodes, n_edges)
    key = (NP, CPB)
    if key not in _CACHE:
        _CACHE[key] = _build(NP, NBLK, BPC, CPB)
    nc = _CACHE[key]
    res = bass_utils.run_bass_kernel_spmd(nc, in_maps, core_ids=list(range(NCORES)),
                                          trace=trace)
    out = np.concatenate([np.asarray(res.results[k]["out"]) for k in range(NCORES)],
                         axis=0)[:n_nodes]
    return np.ascontiguousarray(out.astype(np.float32)), res


def kernel(x, edge_index, W1, att_src1, att_dst1, W2, att_src2, att_dst2):
    out, _ = _run(x, edge_index, W1, att_src1, att_dst1, W2, att_src2, att_dst2,
                  N_NODES, N_EDGES)
    return out
